# revision 71
# baseline (speedup 1.0000x reference)
"""MoE (DeepSeek-style, no gate) SwiGLU kernel for 8 Trainium2 NeuronCores.

Strategy (expert parallelism, per the sharding hint):
  - 16 routed experts sharded 2-per-core across 8 cores.
  - Token dispatch done host-side: for each expert, gather the tokens routed
    to it (topk membership), pad to a common capacity C, and ship the
    pre-transposed activation columns xT[:, tokens] to the owning core.
  - Shared expert tensor-parallel over its inter dim (2816/8 = 352 cols per
    core, zero-padded to 384), computed on all 2048 tokens in 2 chunks.
  - Each core runs the same Bass program (SPMD) on its own shard; the host
    scatter-adds routed outputs and sums shared-expert partials.

Compute layout per phase (one phase = one SwiGLU MLP on a token set):
  phase 1:  h1T = W1p.T @ xT, h3T = W3p.T @ xT   (I on partitions, tokens free)
            h' = silu(h1T) * h3T                  -> bf16 in SBUF
  phase 2:  y[tok_tile] = h'.T @ W2p, scaled per-token by comb weight on
            PSUM eviction (tensor_scalar with a [128,1] per-partition scalar).

All weights/activations are cast to bf16 on host (halves HBM traffic; PE
runs bf16 at 1 cycle/row). PSUM accumulation is fp32; outputs are fp32.
Host pre-tiles every tensor so that every DMA is fully contiguous.
"""

import numpy as np
import ml_dtypes

import concourse.bass as bass
import concourse.bacc as bacc
import concourse.mybir as mybir
import concourse.tile as tile
from concourse.bass_utils import run_bass_kernel_spmd

BF16 = ml_dtypes.bfloat16
F32 = np.float32
P = 128
NSZ = 512  # PSUM bank free size (fp32)

FULL_CFG = dict(
    ncores=8,
    T=2048,
    D=2048,
    E=16,
    I_E=1408,
    sh_half=1408,    # shared expert sharded 2 (inter) x ncores/2 (tokens)
    d_out=2048,
)


def _xgroup(kd):
    """xt k-tiles per DMA (HWDGE executes DMAs serially; batch them)."""
    return 4 if kd % 4 == 0 else (2 if kd % 2 == 0 else 1)


def _derived(cfg):
    nt = max(1, cfg["ncores"] // 2)
    return dict(
        epc=cfg["E"] // cfg["ncores"],
        kd=cfg["D"] // P,
        it_r=cfg["I_E"] // P,
        it_s=cfg["sh_half"] // P,
        n_tok_shards=nt,
        sh_tok=cfg["T"] // nt,
    )


def _emit_phase(nc, pools, xt_dram, w13_dram, w2_dram, cb_dram,
                out_rows, n_itiles, cp, cfg, ph, xt_act_ring=False):
    """One SwiGLU MLP phase over `cp` token columns with `n_itiles` I-tiles.

    Output is TRANSPOSED: out_rows[mt2] is the DRAM destination for model-dim
    tile mt2 ([128, cp] = [D-tile, tokens]), so the token dim streams as the
    matmul free dim at its exact count (no tile padding).
    cb_dram is None for the shared expert (no per-token combine weight);
    otherwise it is the combine weight broadcast to [128, cp].
    """
    kd = _derived(cfg)["kd"]
    d_out = cfg["d_out"]
    dt = mybir.dt.bfloat16
    f32 = mybir.dt.float32

    xtp, wp, hpp, w2p, sp, op, cgp, psA, psY = (
        pools["xt"], pools["w"], pools["hp"], pools["w2"], pools["s"],
        pools["o"], pools["cg"], pools["psA"], pools["psY"])

    # DMA order: first W1/W3 panel (one fused DMA), then xt in groups of 4
    # k-tiles (HWDGE rings execute DMAs serially; batching amortizes the
    # ~0.6us per-DMA cost), then the second panel.
    xg = _xgroup(kd)
    xt_dma = nc.scalar.dma_start if xt_act_ring else nc.sync.dma_start
    wpre = [wp.tile([P, 2, kd, P], dt, tag="w13", name=f"w13_{ph}_0")]
    nc.sync.dma_start(out=wpre[0][:], in_=w13_dram[0])
    xtg = []
    for g in range(kd // xg):
        xge = xtp.tile([P, xg, cp], dt, tag=f"xt_{g}", name=f"xt_{ph}_{g}")
        xt_dma(out=xge[:], in_=xt_dram[g])
        xtg.append(xge)
    if n_itiles > 1:
        w13b = wp.tile([P, 2, kd, P], dt, tag="w13", name=f"w13_{ph}_1")
        nc.sync.dma_start(out=w13b[:], in_=w13_dram[1])
        wpre.append(w13b)

    cbt = None
    if cb_dram is not None:
        cbr = cgp.tile([P, cp], f32, tag="cbr", name=f"cbr_{ph}")
        nc.sync.dma_start(out=cbr[:], in_=cb_dram[:])
        # Bounce through DVE so the per-tile eviction muls below need only
        # the PE wait (DVE has already observed the cb DMA here).
        cbt = cgp.tile([P, cp], f32, tag="cb", name=f"cb_{ph}")
        nc.vector.tensor_copy(cbt[:], cbr[:])

    # ---- phase 1: h' = silu(xW1) * (xW3), transposed layout [I, tokens] ----
    hp = []
    for m in range(n_itiles):
        if m < len(wpre):
            w13t = wpre[m]
        else:
            w13t = wp.tile([P, 2, kd, P], dt, tag="w13", name=f"w13_{ph}_{m}")
            nc.sync.dma_start(out=w13t[:], in_=w13_dram[m])
        hpm = hpp.tile([P, cp], dt, tag=f"hp_{m}", name=f"hp_{ph}_{m}")
        for n0 in range(0, cp, NSZ):
            nsz = min(NSZ, cp - n0)
            p1 = psA.tile([P, nsz], f32, tag="p1", name=f"p1_{ph}_{m}_{n0}")
            p3 = psA.tile([P, nsz], f32, tag="p3", name=f"p3_{ph}_{m}_{n0}")
            for kt in range(kd):
                nc.tensor.matmul(p1[:], w13t[:, 0, kt, :],
                                 xtg[kt // xg][:, kt % xg, n0:n0 + nsz],
                                 start=(kt == 0), stop=(kt == kd - 1))
            for kt in range(kd):
                nc.tensor.matmul(p3[:], w13t[:, 1, kt, :],
                                 xtg[kt // xg][:, kt % xg, n0:n0 + nsz],
                                 start=(kt == 0), stop=(kt == kd - 1))
            # silu(h1)*h3 = sigmoid(h1)*h3*h1 (CoreSim has no Silu LUT).
            # Both ACT ops read PSUM (wait on PE); both DVE ops then wait on
            # a single engine each — the DVE TensorTensor encoding only has
            # room for one sync-wait command.
            s = sp.tile([P, nsz], f32, tag="s", name=f"s_{ph}_{m}_{n0}")
            nc.scalar.activation(s[:], p1[:],
                                 mybir.ActivationFunctionType.Sigmoid)
            c3 = sp.tile([P, nsz], f32, tag="c3", name=f"c3_{ph}_{m}_{n0}")
            nc.scalar.copy(c3[:], p3[:])
            t = sp.tile([P, nsz], f32, tag="t", name=f"t_{ph}_{m}_{n0}")
            nc.vector.tensor_mul(t[:], s[:], c3[:])
            nc.vector.tensor_mul(hpm[:, n0:n0 + nsz], t[:], p1[:])
        hp.append(hpm)

    # ---- phase 2: out[tok] = comb * (h'.T @ W2) ----
    w2t = []
    for kt in range(n_itiles):
        w = w2p.tile([P, d_out], dt, tag=f"w2_{kt}", name=f"w2_{ph}_{kt}")
        nc.sync.dma_start(out=w[:], in_=w2_dram[kt])
        w2t.append(w)

    # Cycle PSUM tags so phase 2 rotates through all 8 banks (phase 1's
    # p1/p3 slots are idle here).
    ps2 = [(psY, "py"), (psY, "py"), (psY, "py"), (psY, "py"),
           (psA, "p1"), (psA, "p1"), (psA, "p3"), (psA, "p3")]
    idx = 0
    for mt2 in range(kd):
        osb = op.tile([P, cp], f32, tag="osb", name=f"osb_{ph}_{mt2}")
        for n0 in range(0, cp, NSZ):
            nn = min(NSZ, cp - n0)
            pool, ptag = ps2[idx % len(ps2)]
            idx += 1
            py = pool.tile([P, nn], f32, tag=ptag, name=f"py_{ph}_{mt2}_{n0}")
            for kt in range(n_itiles):
                nc.tensor.matmul(py[:], w2t[kt][:, mt2 * P:(mt2 + 1) * P],
                                 hp[kt][:, n0:n0 + nn],
                                 start=(kt == 0), stop=(kt == n_itiles - 1))
            if cbt is not None:
                nc.vector.tensor_mul(osb[:, n0:n0 + nn], py[:],
                                     cbt[:, n0:n0 + nn])
            elif idx % 2:
                # Shared-expert evictions alternate DVE/ACT: with only
                # n_itiles=3 matmuls per PSUM group, a single engine's copy
                # throughput (~680ns per [128,512] fp32-from-PSUM) can't keep
                # up with PE (~645ns/group) and PE stalls on bank recycling.
                nc.vector.tensor_copy(osb[:, n0:n0 + nn], py[:])
            else:
                nc.scalar.copy(osb[:, n0:n0 + nn], py[:])
        nc.sync.dma_start(out=out_rows[mt2], in_=osb[:])


def build_program(Cs, cfg):
    """Build the per-core Bass program. Cs[j] = token capacity of routed
    expert slot j (experts are sorted by routed-token count into slots, so
    each slot's capacity matches its own worst case)."""
    nc = bacc.Bacc()
    dt = mybir.dt.bfloat16
    f32 = mybir.dt.float32
    dv = _derived(cfg)
    epc, kd, it_r, it_s = dv["epc"], dv["kd"], dv["it_r"], dv["it_s"]
    sh_tok = dv["sh_tok"]
    d_out = cfg["d_out"]

    xg = _xgroup(kd)
    ins = {}
    for j in range(epc):
        ins[f"xt{j}"] = nc.dram_tensor(f"xt{j}", [kd // xg, P, xg, Cs[j]], dt, kind="ExternalInput")
        ins[f"w13_{j}"] = nc.dram_tensor(f"w13_{j}", [it_r, P, 2, kd, P], dt, kind="ExternalInput")
        ins[f"w2_{j}"] = nc.dram_tensor(f"w2_{j}", [it_r, P, d_out], dt, kind="ExternalInput")
        ins[f"cb{j}"] = nc.dram_tensor(f"cb{j}", [P, Cs[j]], f32, kind="ExternalInput")
    ins["xts"] = nc.dram_tensor("xts", [kd // xg, P, xg, sh_tok], dt, kind="ExternalInput")
    ins["ws13"] = nc.dram_tensor("ws13", [it_s, P, 2, kd, P], dt, kind="ExternalInput")
    ins["ws2"] = nc.dram_tensor("ws2", [it_s, P, d_out], dt, kind="ExternalInput")

    # Outputs are transposed: [D-tile, 128, tokens].
    outs = {}
    for j in range(epc):
        outs[f"y{j}"] = nc.dram_tensor(f"y{j}", [kd, P, Cs[j]], f32, kind="ExternalOutput")
    outs["z"] = nc.dram_tensor("z", [kd, P, sh_tok], f32, kind="ExternalOutput")

    with tile.TileContext(nc) as tc:
        with (
            tc.tile_pool(name="xt", bufs=2) as xtp,
            tc.tile_pool(name="w", bufs=3) as wp,
            tc.tile_pool(name="hp", bufs=1) as hpp,
            tc.tile_pool(name="w2", bufs=1) as w2p,
            tc.tile_pool(name="s", bufs=3) as sp,
            tc.tile_pool(name="o", bufs=3) as op,
            tc.tile_pool(name="cg", bufs=2) as cgp,
            tc.tile_pool(name="psA", bufs=2, space="PSUM") as psA,
            tc.tile_pool(name="psY", bufs=4, space="PSUM") as psY,
        ):
            pools = dict(xt=xtp, w=wp, hp=hpp, w2=w2p, s=sp, o=op, cg=cgp,
                         psA=psA, psY=psY)
            for j in range(epc):
                # First phase's xt rides the ACT HWDGE ring: at kernel start
                # ACT has no compute queued, so both rings fill in parallel.
                _emit_phase(nc, pools, ins[f"xt{j}"], ins[f"w13_{j}"],
                            ins[f"w2_{j}"], ins[f"cb{j}"],
                            [outs[f"y{j}"][mt2] for mt2 in range(kd)],
                            it_r, Cs[j], cfg, ph=f"e{j}",
                            xt_act_ring=(j == 0))
            _emit_phase(nc, pools, ins["xts"], ins["ws13"],
                        ins["ws2"], None,
                        [outs["z"][mt2] for mt2 in range(kd)],
                        it_s, sh_tok, cfg, ph="s")
    nc.compile()
    return nc


def _panelize_w13(w1, w3, itiles):
    """(D, I) x2 -> (itiles, 128, 2, kd, 128): one contiguous DMA per I-tile
    panel carrying both the W1 and W3 slices."""
    dd, ii = w1.shape
    p1 = w1.reshape(dd // P, P, itiles, P).transpose(2, 1, 0, 3)
    p3 = w3.reshape(dd // P, P, itiles, P).transpose(2, 1, 0, 3)
    return np.ascontiguousarray(np.stack([p1, p3], axis=2))


def prep(x, weights, indices, W1, W3, W2, Ws1, Ws3, Ws2, cfg, force_C=None):
    """Host-side dispatch: shard/gather/pad/cast/pre-tile all inputs."""
    T, D, E = cfg["T"], cfg["D"], cfg["E"]
    dv = _derived(cfg)
    epc, kd, it_r, it_s = dv["epc"], dv["kd"], dv["it_r"], dv["it_s"]
    nt, sh_tok = dv["n_tok_shards"], dv["sh_tok"]
    sh_half = cfg["sh_half"]

    xf = np.asarray(x, F32).reshape(T, D)
    wts = np.asarray(weights, F32)
    idx = np.asarray(indices).astype(np.int64)
    W1 = np.asarray(W1, F32)
    W3 = np.asarray(W3, F32)
    W2 = np.asarray(W2, F32)
    Ws1 = np.asarray(Ws1, F32)
    Ws3 = np.asarray(Ws3, F32)
    Ws2 = np.asarray(Ws2, F32)

    # Per-(token, expert) combine weight; duplicate expert ids accumulate.
    comb = np.zeros((T, E), F32)
    np.add.at(comb, (np.arange(T)[:, None], idx), wts)

    # Token dispatch (host-side all-to-all): gather token ids per expert.
    tok_lists = [np.nonzero((idx == e).any(axis=1))[0] for e in range(E)]
    counts = [len(t) for t in tok_lists]
    # Sort experts by routed-token count into the `epc` phase slots so each
    # slot's capacity is only as large as its own worst expert.
    order = np.argsort(counts)[::-1]
    eslot = order.reshape(epc, cfg["ncores"])  # eslot[j][c] = expert id
    if force_C is None:
        Cs = [int(max(NSZ, -(-max(counts[e] for e in eslot[j]) // 16) * 16))
              for j in range(epc)]
    else:
        Cs = [force_C] * epc
    for j in range(epc):
        assert Cs[j] >= max(counts[e] for e in eslot[j])

    xT16 = np.ascontiguousarray(xf.T).astype(BF16)  # (D, T)
    xg = _xgroup(kd)

    def _xt_layout(cols):
        # (D, n) -> (kd//xg, P, xg, n): one contiguous DMA per k-tile group.
        n = cols.shape[1]
        return np.ascontiguousarray(
            cols.reshape(kd // xg, xg, P, n).swapaxes(1, 2))

    in_maps = []
    for c in range(cfg["ncores"]):
        m = {}
        for j in range(epc):
            e = int(eslot[j][c])
            toks = tok_lists[e]
            tpad = np.zeros(Cs[j], np.int64)
            tpad[:counts[e]] = toks
            m[f"xt{j}"] = _xt_layout(xT16[:, tpad])
            m[f"w13_{j}"] = _panelize_w13(W1[e], W3[e], it_r).astype(BF16)
            m[f"w2_{j}"] = np.ascontiguousarray(W2[e].reshape(it_r, P, cfg["d_out"])).astype(BF16)
            cg = np.zeros(Cs[j], F32)
            cg[:counts[e]] = comb[toks, e]
            m[f"cb{j}"] = np.ascontiguousarray(np.broadcast_to(cg, (P, Cs[j])))
        # Shared expert: 2-way inter split x (ncores/2)-way token split.
        h, q = divmod(c, nt)
        m["xts"] = _xt_layout(xT16[:, q * sh_tok:(q + 1) * sh_tok])
        m["ws13"] = _panelize_w13(Ws1[:, h * sh_half:(h + 1) * sh_half],
                                  Ws3[:, h * sh_half:(h + 1) * sh_half],
                                  it_s).astype(BF16)
        m["ws2"] = np.ascontiguousarray(
            Ws2[h * sh_half:(h + 1) * sh_half].reshape(it_s, P, cfg["d_out"])).astype(BF16)
        in_maps.append(m)

    meta = dict(tok_lists=tok_lists, counts=counts, Cs=Cs, eslot=eslot)
    return in_maps, meta


def combine(results, meta, cfg):
    """Host-side unshard: sum shared partials, scatter-add routed outputs."""
    T, D = cfg["T"], cfg["d_out"]
    dv = _derived(cfg)
    epc, nt, sh_tok = dv["epc"], dv["n_tok_shards"], dv["sh_tok"]
    out = np.zeros((T, D), F32)
    for c in range(cfg["ncores"]):
        r = results[c]
        q = c % nt
        # z: (kd, 128, sh_tok) -> (D, sh_tok); two cores (inter halves) add
        # into the same token quarter.
        out[q * sh_tok:(q + 1) * sh_tok] += r["z"].reshape(D, sh_tok).T
        for j in range(epc):
            e = int(meta["eslot"][j][c])
            yt = r[f"y{j}"].reshape(D, -1)  # (D, Cs[j])
            out[meta["tok_lists"][e]] += yt.T[:meta["counts"][e]]
    return out


# Test-harness knobs (kernel() callers get no-trace defaults).
TRACE = False
TMPDIR = None
LAST_RESULT = None


def kernel(x, weights, indices, W1, W3, W2, Ws1, Ws3, Ws2):
    global LAST_RESULT
    cfg = FULL_CFG
    in_maps, meta = prep(x, weights, indices, W1, W3, W2,
                         Ws1, Ws3, Ws2, cfg)
    nc = build_program(meta["Cs"], cfg)
    res = run_bass_kernel_spmd(nc, in_maps, core_ids=list(range(cfg["ncores"])),
                               trace=TRACE, tmpdir=TMPDIR)
    LAST_RESULT = res
    out = combine(res.results, meta, cfg)
    return out.reshape(1, cfg["T"], cfg["D"]).astype(F32)


# revision 72
# speedup vs baseline: 1.0173x; 1.0173x over previous
"""MoE (DeepSeek-style, no gate) SwiGLU kernel for 8 Trainium2 NeuronCores.

Strategy (expert parallelism, per the sharding hint):
  - 16 routed experts sharded 2-per-core across 8 cores.
  - Token dispatch done host-side: for each expert, gather the tokens routed
    to it (topk membership), pad to a common capacity C, and ship the
    pre-transposed activation columns xT[:, tokens] to the owning core.
  - Shared expert tensor-parallel over its inter dim (2816/8 = 352 cols per
    core, zero-padded to 384), computed on all 2048 tokens in 2 chunks.
  - Each core runs the same Bass program (SPMD) on its own shard; the host
    scatter-adds routed outputs and sums shared-expert partials.

Compute layout per phase (one phase = one SwiGLU MLP on a token set):
  phase 1:  h1T = W1p.T @ xT, h3T = W3p.T @ xT   (I on partitions, tokens free)
            h' = silu(h1T) * h3T                  -> bf16 in SBUF
  phase 2:  y[tok_tile] = h'.T @ W2p, scaled per-token by comb weight on
            PSUM eviction (tensor_scalar with a [128,1] per-partition scalar).

All weights/activations are cast to bf16 on host (halves HBM traffic; PE
runs bf16 at 1 cycle/row). PSUM accumulation is fp32; outputs are fp32.
Host pre-tiles every tensor so that every DMA is fully contiguous.
"""

import numpy as np
import ml_dtypes

import concourse.bass as bass
import concourse.bacc as bacc
import concourse.mybir as mybir
import concourse.tile as tile
from concourse.bass_utils import run_bass_kernel_spmd

BF16 = ml_dtypes.bfloat16
F32 = np.float32
P = 128
NSZ = 512  # PSUM bank free size (fp32)

FULL_CFG = dict(
    ncores=8,
    T=2048,
    D=2048,
    E=16,
    I_E=1408,
    sh_half=1408,    # shared expert sharded 2 (inter) x ncores/2 (tokens)
    d_out=2048,
)


def _xgroup(kd):
    """xt k-tiles per DMA (HWDGE executes DMAs serially; batch them)."""
    return 4 if kd % 4 == 0 else (2 if kd % 2 == 0 else 1)


def _derived(cfg):
    nt = max(1, cfg["ncores"] // 2)
    return dict(
        epc=cfg["E"] // cfg["ncores"],
        kd=cfg["D"] // P,
        it_r=cfg["I_E"] // P,
        it_s=cfg["sh_half"] // P,
        n_tok_shards=nt,
        sh_tok=cfg["T"] // nt,
    )


def _emit_phase(nc, pools, xt_dram, w13_dram, w2_dram, cb_dram,
                out_rows, n_itiles, cp, cfg, ph, xt_act_ring=False):
    """One SwiGLU MLP phase over `cp` token columns with `n_itiles` I-tiles.

    Output is TRANSPOSED: out_rows[mt2] is the DRAM destination for model-dim
    tile mt2 ([128, cp] = [D-tile, tokens]), so the token dim streams as the
    matmul free dim at its exact count (no tile padding).
    cb_dram is None for the shared expert (no per-token combine weight);
    otherwise it is the combine weight broadcast to [128, cp].
    """
    kd = _derived(cfg)["kd"]
    d_out = cfg["d_out"]
    dt = mybir.dt.bfloat16
    f32 = mybir.dt.float32

    xtp, wp, hpp, w2p, sp, op, cgp, psA, psY = (
        pools["xt"], pools["w"], pools["hp"], pools["w2"], pools["s"],
        pools["o"], pools["cg"], pools["psA"], pools["psY"])

    # DMA order: first W1/W3 panel (one fused DMA), then xt in groups of 4
    # k-tiles (HWDGE rings execute DMAs serially; batching amortizes the
    # ~0.6us per-DMA cost), then the second panel.
    xg = _xgroup(kd)
    xt_dma = nc.scalar.dma_start if xt_act_ring else nc.sync.dma_start
    wpre = [wp.tile([P, 2, kd, P], dt, tag="w13", name=f"w13_{ph}_0")]
    nc.sync.dma_start(out=wpre[0][:], in_=w13_dram[0])
    xtg = []
    for g in range(kd // xg):
        xge = xtp.tile([P, xg, cp], dt, tag=f"xt_{g}", name=f"xt_{ph}_{g}")
        xt_dma(out=xge[:], in_=xt_dram[g])
        xtg.append(xge)
    if n_itiles > 1:
        w13b = wp.tile([P, 2, kd, P], dt, tag="w13", name=f"w13_{ph}_1")
        nc.sync.dma_start(out=w13b[:], in_=w13_dram[1])
        wpre.append(w13b)

    cbt = None
    if cb_dram is not None:
        cbr = cgp.tile([P, cp], f32, tag="cbr", name=f"cbr_{ph}")
        nc.sync.dma_start(out=cbr[:], in_=cb_dram[:])
        # Bounce through DVE so the per-tile eviction muls below need only
        # the PE wait (DVE has already observed the cb DMA here).
        cbt = cgp.tile([P, cp], f32, tag="cb", name=f"cb_{ph}")
        nc.vector.tensor_copy(cbt[:], cbr[:])

    # ---- phase 1: h' = silu(xW1) * (xW3), transposed layout [I, tokens] ----
    hp = []
    for m in range(n_itiles):
        if m < len(wpre):
            w13t = wpre[m]
        else:
            w13t = wp.tile([P, 2, kd, P], dt, tag="w13", name=f"w13_{ph}_{m}")
            nc.sync.dma_start(out=w13t[:], in_=w13_dram[m])
        hpm = hpp.tile([P, cp], dt, tag=f"hp_{m}", name=f"hp_{ph}_{m}")
        for n0 in range(0, cp, NSZ):
            nsz = min(NSZ, cp - n0)
            p1 = psA.tile([P, nsz], f32, tag="p1", name=f"p1_{ph}_{m}_{n0}")
            p3 = psA.tile([P, nsz], f32, tag="p3", name=f"p3_{ph}_{m}_{n0}")
            for kt in range(kd):
                nc.tensor.matmul(p1[:], w13t[:, 0, kt, :],
                                 xtg[kt // xg][:, kt % xg, n0:n0 + nsz],
                                 start=(kt == 0), stop=(kt == kd - 1))
            for kt in range(kd):
                nc.tensor.matmul(p3[:], w13t[:, 1, kt, :],
                                 xtg[kt // xg][:, kt % xg, n0:n0 + nsz],
                                 start=(kt == 0), stop=(kt == kd - 1))
            # silu(h1)*h3 = sigmoid(h1)*h3*h1 (CoreSim has no Silu LUT).
            # Both ACT ops read PSUM (wait on PE); both DVE ops then wait on
            # a single engine each — the DVE TensorTensor encoding only has
            # room for one sync-wait command.
            s = sp.tile([P, nsz], f32, tag="s", name=f"s_{ph}_{m}_{n0}")
            nc.scalar.activation(s[:], p1[:],
                                 mybir.ActivationFunctionType.Sigmoid)
            c3 = sp.tile([P, nsz], f32, tag="c3", name=f"c3_{ph}_{m}_{n0}")
            nc.scalar.copy(c3[:], p3[:])
            t = sp.tile([P, nsz], f32, tag="t", name=f"t_{ph}_{m}_{n0}")
            nc.vector.tensor_mul(t[:], s[:], c3[:])
            nc.vector.tensor_mul(hpm[:, n0:n0 + nsz], t[:], p1[:])
        hp.append(hpm)

    # ---- phase 2: out[tok] = comb * (h'.T @ W2) ----
    w2t = []
    for kt in range(n_itiles):
        w = w2p.tile([P, d_out], dt, tag=f"w2_{kt}", name=f"w2_{ph}_{kt}")
        nc.sync.dma_start(out=w[:], in_=w2_dram[kt])
        w2t.append(w)

    # Cycle PSUM tags so phase 2 rotates through all 8 banks (phase 1's
    # p1/p3 slots are idle here).
    ps2 = [(psY, "py"), (psY, "py"), (psY, "py"), (psY, "py"),
           (psA, "p1"), (psA, "p1"), (psA, "p3"), (psA, "p3")]
    idx = 0
    for mt2 in range(kd):
        osb = op.tile([P, cp], f32, tag="osb", name=f"osb_{ph}_{mt2}")
        for n0 in range(0, cp, NSZ):
            nn = min(NSZ, cp - n0)
            pool, ptag = ps2[idx % len(ps2)]
            idx += 1
            py = pool.tile([P, nn], f32, tag=ptag, name=f"py_{ph}_{mt2}_{n0}")
            for kt in range(n_itiles):
                nc.tensor.matmul(py[:], w2t[kt][:, mt2 * P:(mt2 + 1) * P],
                                 hp[kt][:, n0:n0 + nn],
                                 start=(kt == 0), stop=(kt == n_itiles - 1))
            if cbt is not None:
                nc.vector.tensor_mul(osb[:, n0:n0 + nn], py[:],
                                     cbt[:, n0:n0 + nn])
            elif idx % 2:
                # Shared-expert evictions alternate DVE/ACT: with only
                # n_itiles=3 matmuls per PSUM group, a single engine's copy
                # throughput (~680ns per [128,512] fp32-from-PSUM) can't keep
                # up with PE (~645ns/group) and PE stalls on bank recycling.
                nc.vector.tensor_copy(osb[:, n0:n0 + nn], py[:])
            else:
                nc.scalar.copy(osb[:, n0:n0 + nn], py[:])
        nc.sync.dma_start(out=out_rows[mt2], in_=osb[:])


def build_program(Cs, cfg):
    """Build the per-core Bass program. Cs[j] = token capacity of routed
    expert slot j (experts are sorted by routed-token count into slots, so
    each slot's capacity matches its own worst case)."""
    nc = bacc.Bacc()
    dt = mybir.dt.bfloat16
    f32 = mybir.dt.float32
    dv = _derived(cfg)
    epc, kd, it_r, it_s = dv["epc"], dv["kd"], dv["it_r"], dv["it_s"]
    sh_tok = dv["sh_tok"]
    d_out = cfg["d_out"]

    xg = _xgroup(kd)
    ins = {}
    for j in range(epc):
        ins[f"xt{j}"] = nc.dram_tensor(f"xt{j}", [kd // xg, P, xg, Cs[j]], dt, kind="ExternalInput")
        ins[f"w13_{j}"] = nc.dram_tensor(f"w13_{j}", [it_r, P, 2, kd, P], dt, kind="ExternalInput")
        ins[f"w2_{j}"] = nc.dram_tensor(f"w2_{j}", [it_r, P, d_out], dt, kind="ExternalInput")
        ins[f"cb{j}"] = nc.dram_tensor(f"cb{j}", [P, Cs[j]], f32, kind="ExternalInput")
    ins["xts"] = nc.dram_tensor("xts", [kd // xg, P, xg, sh_tok], dt, kind="ExternalInput")
    ins["ws13"] = nc.dram_tensor("ws13", [it_s, P, 2, kd, P], dt, kind="ExternalInput")
    ins["ws2"] = nc.dram_tensor("ws2", [it_s, P, d_out], dt, kind="ExternalInput")

    # Outputs are transposed: [D-tile, 128, tokens].
    outs = {}
    for j in range(epc):
        outs[f"y{j}"] = nc.dram_tensor(f"y{j}", [kd, P, Cs[j]], f32, kind="ExternalOutput")
    outs["z"] = nc.dram_tensor("z", [kd, P, sh_tok], f32, kind="ExternalOutput")

    with tile.TileContext(nc) as tc:
        with (
            tc.tile_pool(name="xt", bufs=2) as xtp,
            tc.tile_pool(name="w", bufs=3) as wp,
            tc.tile_pool(name="hp", bufs=1) as hpp,
            tc.tile_pool(name="w2", bufs=1) as w2p,
            tc.tile_pool(name="s", bufs=3) as sp,
            tc.tile_pool(name="o", bufs=3) as op,
            tc.tile_pool(name="cg", bufs=2) as cgp,
            tc.tile_pool(name="psA", bufs=2, space="PSUM") as psA,
            tc.tile_pool(name="psY", bufs=4, space="PSUM") as psY,
        ):
            pools = dict(xt=xtp, w=wp, hp=hpp, w2=w2p, s=sp, o=op, cg=cgp,
                         psA=psA, psY=psY)
            # All DMAs stay on the SP HWDGE ring: experiments routing xt
            # loads via the ACT ring consistently regressed (the DMAs block
            # the ACT sigmoid stream queued behind them).
            for j in range(epc):
                _emit_phase(nc, pools, ins[f"xt{j}"], ins[f"w13_{j}"],
                            ins[f"w2_{j}"], ins[f"cb{j}"],
                            [outs[f"y{j}"][mt2] for mt2 in range(kd)],
                            it_r, Cs[j], cfg, ph=f"e{j}")
            _emit_phase(nc, pools, ins["xts"], ins["ws13"],
                        ins["ws2"], None,
                        [outs["z"][mt2] for mt2 in range(kd)],
                        it_s, sh_tok, cfg, ph="s")
    nc.compile()
    return nc


def _panelize_w13(w1, w3, itiles):
    """(D, I) x2 -> (itiles, 128, 2, kd, 128): one contiguous DMA per I-tile
    panel carrying both the W1 and W3 slices."""
    dd, ii = w1.shape
    p1 = w1.reshape(dd // P, P, itiles, P).transpose(2, 1, 0, 3)
    p3 = w3.reshape(dd // P, P, itiles, P).transpose(2, 1, 0, 3)
    return np.ascontiguousarray(np.stack([p1, p3], axis=2))


def prep(x, weights, indices, W1, W3, W2, Ws1, Ws3, Ws2, cfg, force_C=None):
    """Host-side dispatch: shard/gather/pad/cast/pre-tile all inputs."""
    T, D, E = cfg["T"], cfg["D"], cfg["E"]
    dv = _derived(cfg)
    epc, kd, it_r, it_s = dv["epc"], dv["kd"], dv["it_r"], dv["it_s"]
    nt, sh_tok = dv["n_tok_shards"], dv["sh_tok"]
    sh_half = cfg["sh_half"]

    xf = np.asarray(x, F32).reshape(T, D)
    wts = np.asarray(weights, F32)
    idx = np.asarray(indices).astype(np.int64)
    W1 = np.asarray(W1, F32)
    W3 = np.asarray(W3, F32)
    W2 = np.asarray(W2, F32)
    Ws1 = np.asarray(Ws1, F32)
    Ws3 = np.asarray(Ws3, F32)
    Ws2 = np.asarray(Ws2, F32)

    # Per-(token, expert) combine weight; duplicate expert ids accumulate.
    comb = np.zeros((T, E), F32)
    np.add.at(comb, (np.arange(T)[:, None], idx), wts)

    # Token dispatch (host-side all-to-all): gather token ids per expert.
    tok_lists = [np.nonzero((idx == e).any(axis=1))[0] for e in range(E)]
    counts = [len(t) for t in tok_lists]
    # Sort experts by routed-token count into the `epc` phase slots so each
    # slot's capacity is only as large as its own worst expert.
    order = np.argsort(counts)[::-1]
    eslot = order.reshape(epc, cfg["ncores"])  # eslot[j][c] = expert id
    if force_C is None:
        Cs = [int(max(NSZ, -(-max(counts[e] for e in eslot[j]) // 16) * 16))
              for j in range(epc)]
    else:
        Cs = [force_C] * epc
    for j in range(epc):
        assert Cs[j] >= max(counts[e] for e in eslot[j])

    xT16 = np.ascontiguousarray(xf.T).astype(BF16)  # (D, T)
    xg = _xgroup(kd)

    def _xt_layout(cols):
        # (D, n) -> (kd//xg, P, xg, n): one contiguous DMA per k-tile group.
        n = cols.shape[1]
        return np.ascontiguousarray(
            cols.reshape(kd // xg, xg, P, n).swapaxes(1, 2))

    in_maps = []
    for c in range(cfg["ncores"]):
        m = {}
        for j in range(epc):
            e = int(eslot[j][c])
            toks = tok_lists[e]
            tpad = np.zeros(Cs[j], np.int64)
            tpad[:counts[e]] = toks
            m[f"xt{j}"] = _xt_layout(xT16[:, tpad])
            m[f"w13_{j}"] = _panelize_w13(W1[e], W3[e], it_r).astype(BF16)
            m[f"w2_{j}"] = np.ascontiguousarray(W2[e].reshape(it_r, P, cfg["d_out"])).astype(BF16)
            cg = np.zeros(Cs[j], F32)
            cg[:counts[e]] = comb[toks, e]
            m[f"cb{j}"] = np.ascontiguousarray(np.broadcast_to(cg, (P, Cs[j])))
        # Shared expert: 2-way inter split x (ncores/2)-way token split.
        h, q = divmod(c, nt)
        m["xts"] = _xt_layout(xT16[:, q * sh_tok:(q + 1) * sh_tok])
        m["ws13"] = _panelize_w13(Ws1[:, h * sh_half:(h + 1) * sh_half],
                                  Ws3[:, h * sh_half:(h + 1) * sh_half],
                                  it_s).astype(BF16)
        m["ws2"] = np.ascontiguousarray(
            Ws2[h * sh_half:(h + 1) * sh_half].reshape(it_s, P, cfg["d_out"])).astype(BF16)
        in_maps.append(m)

    meta = dict(tok_lists=tok_lists, counts=counts, Cs=Cs, eslot=eslot)
    return in_maps, meta


def combine(results, meta, cfg):
    """Host-side unshard: sum shared partials, scatter-add routed outputs."""
    T, D = cfg["T"], cfg["d_out"]
    dv = _derived(cfg)
    epc, nt, sh_tok = dv["epc"], dv["n_tok_shards"], dv["sh_tok"]
    out = np.zeros((T, D), F32)
    for c in range(cfg["ncores"]):
        r = results[c]
        q = c % nt
        # z: (kd, 128, sh_tok) -> (D, sh_tok); two cores (inter halves) add
        # into the same token quarter.
        out[q * sh_tok:(q + 1) * sh_tok] += r["z"].reshape(D, sh_tok).T
        for j in range(epc):
            e = int(meta["eslot"][j][c])
            yt = r[f"y{j}"].reshape(D, -1)  # (D, Cs[j])
            out[meta["tok_lists"][e]] += yt.T[:meta["counts"][e]]
    return out


# Test-harness knobs (kernel() callers get no-trace defaults).
TRACE = False
TMPDIR = None
LAST_RESULT = None


def kernel(x, weights, indices, W1, W3, W2, Ws1, Ws3, Ws2):
    global LAST_RESULT
    cfg = FULL_CFG
    in_maps, meta = prep(x, weights, indices, W1, W3, W2,
                         Ws1, Ws3, Ws2, cfg)
    nc = build_program(meta["Cs"], cfg)
    res = run_bass_kernel_spmd(nc, in_maps, core_ids=list(range(cfg["ncores"])),
                               trace=TRACE, tmpdir=TMPDIR)
    LAST_RESULT = res
    out = combine(res.results, meta, cfg)
    return out.reshape(1, cfg["T"], cfg["D"]).astype(F32)
